# revision 13
# baseline (speedup 1.0000x reference)
"""Trainium2 Bass kernel for a causal-attention-like module (fp8 DoubleRow).

Math (reassociated from the reference nn.Module):
    dist[i,j] = sqrt(max(|T_i|^2 + |T_j|^2 - 2 T_i.T_j, 0) + 1e-8)
    scale_i   = 1 / (1 + mean_j dist[i,j])
    Q2        = (H Wq^T + bq) Wk / sqrt(d)         # bk cancels inside softmax
    E[i,j]    = exp(Q2[i,:] . H[j,:] - 3)          # -3 shift cancels in rownorm
    G         = E @ H                              # unnormalized
    out       = ((G / rowsum(E)) Wv^T + bv) * scale @ Wo^T + bo

Sharding: rows of H/T (i dimension) split across 8 cores, 1024 rows each;
H (both orientations) and the small dim x dim weights replicated.

Performance shape: the three N*R*D matmuls (distance, logits, E@H) plus the
rowsum run as fp8e4 DoubleRow matmuls (two 128-deep contraction planes per
instruction, PE double-pumped). Operand layout is [128, plane, X] so a
[:, 2u:2u+2, x] slice gives the [K,2,M] shape DoubleRow expects. PSUM banks
alternate between consecutive matmuls (same-bank successors serialize).
fp8 accuracy safeguards:
  - xx augmentation is centered at 512 and carried in two rows (value +
    residual) so e4m3's 3-bit mantissa does not perturb the distance scale;
  - Q2 is scaled by 16 into fp8's normal range; exp() applies scale 1/16
    and bias -3 so E stays well under the TRN fp8e4 max of 240;
  - the small dim x dim projection chains stay bf16.
"""

import math
import os
import sys

import numpy as np

for _p in ("/opt/trn_rl_repo", "/root/.axon_site", "/root/.axon_site/_ro/trn_rl_repo"):
    if os.path.isdir(_p) and _p not in sys.path:
        sys.path.append(_p)

import ml_dtypes

import concourse.bass as bass
import concourse.mybir as mybir
import concourse.tile as tile
from concourse import bacc, bass_utils

N = 8192          # total rows
D = 512           # feature dim
NCORES = 8
R = N // NCORES   # rows per core (1024)
P = 128           # partitions
KT = D // P       # 4 contraction planes
CH = 512          # free-dim chunk (one PSUM bank of f32)
NJC = N // CH     # 16 j-chunks
NJT = N // P      # 64 j-tiles
NPAIR = NJT // 2  # 32 j-tile pairs (DoubleRow granularity)
NIC = R // CH     # 2 i-chunks
NIT = R // P      # 8 i-tiles
JG = 2            # j-chunks per distance group (rotating PSUM banks)
NG = NJC // JG    # 8 distance groups
BF = mybir.dt.bfloat16
F32 = mybir.dt.float32
F8 = mybir.dt.float8e4
DR = mybir.MatmulPerfMode.DoubleRow
AF = mybir.ActivationFunctionType
ALU = mybir.AluOpType
Q2SCALE = 16.0 / math.sqrt(D)   # Q2 stored as 16*Q2_true in fp8
EXPSCALE = 1.0 / 16.0
EXPBIAS = -4.75

bf16 = ml_dtypes.bfloat16
f8e4 = ml_dtypes.float8_e4m3


def _emit(tc, io):
    nc = tc.nc
    from contextlib import ExitStack

    with ExitStack() as ctx:
        const = ctx.enter_context(tc.tile_pool(name="const", bufs=1))
        psum = ctx.enter_context(tc.tile_pool(name="psum", bufs=1, space="PSUM"))
        # attention-phase pools created up front so their SBUF space is
        # carved out early: their first DMAs must not wait on the early
        # pool's release.
        e_pool = ctx.enter_context(tc.tile_pool(name="ep", bufs=4))
        h_pool = ctx.enter_context(tc.tile_pool(name="hp", bufs=4))
        o_pool = ctx.enter_context(tc.tile_pool(name="op", bufs=2))

        # ---- small shared constants ----------------------------------------
        ones_f1 = const.tile([1, P], F32, name="onesf1")
        nc.vector.memset(ones_f1, 1.0)
        ones_b1 = const.tile([1, P], BF, name="onesb1")
        nc.vector.memset(ones_b1, 1.0)
        sixteen_col = const.tile([P, 1], F32, name="sixteencol")
        nc.vector.memset(sixteen_col, 16.0)
        expb_col = const.tile([P, 1], F32, name="expbcol")
        nc.vector.memset(expb_col, EXPBIAS)
        # [128,2,128] fp8 stationary whose first column is ones in both
        # planes: DoubleRow rowsum over a pair of 128-row tiles.
        onesw2 = const.tile([P, 2, P], F8, name="onesw2")
        nc.vector.memset(onesw2, 0.0)
        nc.vector.memset(onesw2[:, :, 0:1], 1.0)
        cst_1r = const.tile([1, R], F8, name="cst1r")
        nc.vector.memset(cst_1r, 1.0)
        cst_m4 = const.tile([1, R], F8, name="cstm4")
        nc.vector.memset(cst_m4, -4.0)
        cst_128 = const.tile([1, JG * CH], F8, name="cst128")
        nc.vector.memset(cst_128, 128.0)

        # ---- long-lived tensors (written early, read late) -----------------
        q2f8 = const.tile([P, KT, R], F8, name="q2f8")
        bv_row = const.tile([1, D], BF, name="bvrow")
        bo_row = const.tile([1, D], BF, name="borow")
        WvT, WoT = [], []
        for m in range(KT):
            wvt_t = const.tile([P, D], BF, name=f"wvt{m}")
            WvT.append(wvt_t)
            wot_t = const.tile([P, D], BF, name=f"wot{m}")
            WoT.append(wot_t)
        # resident fp8 H^T for the logits matmuls; allocated here but DMA'd
        # in pieces during the distance phase so the dist-critical tct/tt
        # loads are never queued behind this 4MB stream.
        ht8 = const.tile([P, KT, N], F8, name="ht8")
        ht8_chunks = [(k, q) for k in range(KT) for q in range(4)]

        def emit_ht8(n):
            for _ in range(n):
                if not ht8_chunks:
                    return
                k, q = ht8_chunks.pop(0)
                nc.sync.dma_start(
                    ht8[:, k:k + 1, q * (N // 4):(q + 1) * (N // 4)],
                    io["HTf8"][k * P:(k + 1) * P,
                               q * (N // 4):(q + 1) * (N // 4)])

        def emit_tailw():
            nc.sync.dma_start(bv_row, io["bvb"][:, :])
            nc.sync.dma_start(bo_row, io["bob"][:, :])
            for m in range(KT):
                nc.sync.dma_start(WvT[m], io["WvTb"][m * P:(m + 1) * P, :])
                nc.sync.dma_start(WoT[m], io["WoTb"][m * P:(m + 1) * P, :])
        GT = [const.tile([P, R], BF, name=f"gt{d_}") for d_ in range(KT)]
        YT = [const.tile([P, R], BF, name=f"yt{m}") for m in range(KT)]
        SNB = const.tile([P, R], F32, name="snb")
        scl_row = const.tile([1, R], F32, name="sclrow")
        scl_b = const.tile([1, R], BF, name="sclb")
        rs_row = const.tile([1, R], F32, name="rsrow")
        sn_row = const.tile([1, R], F32, name="snrow")

        # ---- early phases (scoped SBUF) ------------------------------------
        with tc.tile_pool(name="early", bufs=1) as early:
            tct = early.tile([P, KT, R], F8, name="tct")
            for k in range(KT):
                nc.sync.dma_start(tct[:, k:k + 1, :],
                                  io["TcTf8"][k * P:(k + 1) * P, :])
            # aug operand, 128-deep zero-padded plane0 + zero plane1 so the
            # aug matmul is a normal full-array DoubleRow instruction.
            # plane0 rows: r0=q_i, r1=res_i (pair with moving ones),
            # r2=1, r3=1 (pair with moving q_j, res_j), r4=-4 (pairs with
            # moving 128 -> -512 constant); q+res = -(xx-512)/2.
            aug_lhs = early.tile([P, 2, R], F8, name="auglhs")
            nc.vector.memset(aug_lhs, 0.0)
            nc.sync.dma_start(aug_lhs[2:3, 0:1, :], cst_1r)
            nc.sync.dma_start(aug_lhs[3:4, 0:1, :], cst_1r)
            nc.sync.dma_start(aug_lhs[4:5, 0:1, :], cst_m4)
            dsum = [early.tile([P, NJC], F32, name=f"dsum{it}")
                    for it in range(NIT)]

            with tc.tile_pool(name="sqp", bufs=3) as sq_pool, \
                 tc.tile_pool(name="ttp", bufs=2) as tt_pool, \
                 tc.tile_pool(name="dsp", bufs=3) as dist_pool, \
                 tc.tile_pool(name="augp", bufs=2) as aug_pool:

                # -- xx over this core's own rows -> aug_lhs rows 0/1 --------
                pssc = [psum.tile([P, CH], F32, tag="mm", bufs=3, name="psxxc")
                        for _ in range(NIC)]
                sqcs = [[None] * 2 for _ in range(NIC)]
                for ic in range(NIC):
                    for u in range(2):
                        sqc = sq_pool.tile([P, 2, CH], F8, tag=f"sq{ic}{u}",
                                           name="sqc")
                        for pl in range(2):
                            k = 2 * u + pl
                            nc.vector.tensor_mul(
                                sqc[:, pl:pl + 1, :],
                                tct[:, k:k + 1, ic * CH:(ic + 1) * CH],
                                tct[:, k:k + 1, ic * CH:(ic + 1) * CH])
                        sqcs[ic][u] = sqc
                for u in range(2):
                    for ic in range(NIC):
                        nc.tensor.matmul(pssc[ic], onesw2, sqcs[ic][u],
                                         start=(u == 0), stop=(u == 1),
                                         perf_mode=DR)
                for ic in range(NIC):
                    csl = slice(ic * CH, (ic + 1) * CH)
                    tv = sq_pool.tile([1, CH], F32, tag="tv", bufs=2,
                                      name="tvc")
                    nc.vector.tensor_scalar(tv, pssc[ic][0:1, :], -0.5, 256.0,
                                            op0=ALU.mult, op1=ALU.add)
                    xq = sq_pool.tile([1, CH], F8, tag="xqc", bufs=2,
                                      name="xqc")
                    nc.vector.tensor_copy(xq, tv)
                    xr = sq_pool.tile([1, CH], F8, tag="xrc", bufs=2,
                                      name="xrc")
                    nc.vector.tensor_sub(xr, tv, xq)
                    nc.sync.dma_start(aug_lhs[0:1, 0:1, csl], xq)
                    nc.sync.dma_start(aug_lhs[1:2, 0:1, csl], xr)

                def load_group(jg):
                    tts = []
                    for jj in range(JG):
                        jc = jg * JG + jj
                        tt_t = tt_pool.tile([P, KT, CH], F8, tag=f"tt{jj}",
                                            name="ttd")
                        for k in range(KT):
                            nc.sync.dma_start(
                                tt_t[:, k:k + 1, :],
                                io["TTf8"][k * P:(k + 1) * P,
                                           jc * CH:(jc + 1) * CH])
                        tts.append(tt_t)
                    return tts

                def xx_chain(jg, tts, dve_only=False):
                    # squares on DVE+GPSIMD; xx row via DoubleRow ones-matmul;
                    # value+residual rows land in augg plane0 via SBUF DMA.
                    augg = aug_pool.tile([P, 2, JG * CH], F8, tag="augg",
                                         name="augg")
                    nc.vector.memset(augg, 0.0)
                    nc.vector.memset(augg[0:1, 0:1, :], 1.0)
                    nc.sync.dma_start(augg[1:2, 0:1, :], cst_1r)
                    nc.sync.dma_start(augg[4:5, 0:1, :], cst_128)
                    pxx = [psum.tile([P, CH], F32, tag="mm", bufs=3,
                                     name="psxx") for _ in range(JG)]
                    sqs = [[None] * 2 for _ in range(JG)]
                    for jj in range(JG):
                        for u in range(2):
                            sq = sq_pool.tile([P, 2, CH], F8, tag=f"sq{jj}{u}",
                                              name="sq")
                            for pl in range(2):
                                k = 2 * u + pl
                                eng = (nc.vector if (pl == 0 or dve_only)
                                       else nc.gpsimd)
                                eng.tensor_mul(sq[:, pl:pl + 1, :],
                                               tts[jj][:, k:k + 1, :],
                                               tts[jj][:, k:k + 1, :])
                            sqs[jj][u] = sq
                    for u in range(2):
                        for jj in range(JG):
                            nc.tensor.matmul(pxx[jj], onesw2, sqs[jj][u],
                                             start=(u == 0), stop=(u == 1),
                                             perf_mode=DR)
                    for jj in range(JG):
                        tv = sq_pool.tile([1, CH], F32, tag="tvj", bufs=2,
                                          name="tvj")
                        nc.vector.tensor_scalar(tv, pxx[jj][0:1, :], -0.5,
                                                256.0, op0=ALU.mult,
                                                op1=ALU.add)
                        xq = sq_pool.tile([1, CH], F8, tag="xq", bufs=2,
                                          name="xq")
                        nc.vector.tensor_copy(xq, tv)
                        xr = sq_pool.tile([1, CH], F8, tag="xr", bufs=2,
                                          name="xr")
                        nc.vector.tensor_sub(xr, tv, xq)
                        nc.sync.dma_start(
                            augg[2:3, 0:1, jj * CH:(jj + 1) * CH], xq)
                        nc.sync.dma_start(
                            augg[3:4, 0:1, jj * CH:(jj + 1) * CH], xr)
                    return augg

                PD_TAGS = ["g0", "g1", "g2", "g3", "rowps"]

                def d2_group(jg, tts, augg):
                    for it in range(NIT):
                        # rotate over 5 banks so a bank's sqrt drain has
                        # 2.5 iterations of slack before the PE reuses it
                        pd = [psum.tile([P, CH], F32,
                                        tag=PD_TAGS[(2 * it + jj) % 5],
                                        name=f"psd{jj}") for jj in range(JG)]
                        for u in range(2):
                            for jj in range(JG):
                                nc.tensor.matmul(
                                    pd[jj],
                                    tct[:, 2 * u:2 * u + 2,
                                        it * P:(it + 1) * P],
                                    tts[jj][:, 2 * u:2 * u + 2, :],
                                    start=(u == 0), stop=False, perf_mode=DR)
                        for jj in range(JG):
                            nc.tensor.matmul(
                                pd[jj], aug_lhs[:, :, it * P:(it + 1) * P],
                                augg[:, :, jj * CH:(jj + 1) * CH],
                                start=False, stop=True, perf_mode=DR)
                        for jj in range(JG):
                            jc = jg * JG + jj
                            # sqrt(dist2 + 16) straight from PSUM: the +16
                            # keeps the (fp8-noisy) diagonal positive; the
                            # systematic +8/dist shift is corrected
                            # analytically in the scale computation below.
                            dist_t = dist_pool.tile([P, CH], BF, tag="dist",
                                                    name="distt")
                            nc.scalar.activation(
                                dist_t, pd[jj], AF.Sqrt, scale=-2.0,
                                bias=sixteen_col,
                                accum_out=dsum[it][:, jc:jc + 1])

                tts_cur = load_group(0)
                augg_cur = xx_chain(0, tts_cur, dve_only=True)
                tts_next = load_group(1)
                augg_next = xx_chain(1, tts_next, dve_only=True)
                emit_ht8(2)
                d2_group(0, tts_cur, augg_cur)
                tts_cur, augg_cur = tts_next, augg_next

                # -- Q chain (independent; fills PE while group 2 loads) -----
                with tc.tile_pool(name="qpool", bufs=1) as qpool:
                    HcT, WqT, Wk = [], [], []
                    for k in range(KT):
                        hct_t = qpool.tile([P, R], BF, name=f"hct{k}")
                        nc.sync.dma_start(hct_t,
                                          io["HcTb"][k * P:(k + 1) * P, :])
                        HcT.append(hct_t)
                        wqt_t = qpool.tile([P, D], BF, name=f"wqt{k}")
                        nc.sync.dma_start(wqt_t,
                                          io["WqTb"][k * P:(k + 1) * P, :])
                        WqT.append(wqt_t)
                        wk_t = qpool.tile([P, D], BF, name=f"wk{k}")
                        nc.sync.dma_start(wk_t,
                                          io["Wkb"][k * P:(k + 1) * P, :])
                        Wk.append(wk_t)
                    bq_sb = []
                    for m in range(KT):
                        b_t = qpool.tile([P, 1], F32, name=f"bq{m}")
                        nc.sync.dma_start(b_t, io["bqf"][m * P:(m + 1) * P, :])
                        bq_sb.append(b_t)
                    QT = [qpool.tile([P, R], BF, name=f"qt{m}")
                          for m in range(KT)]
                    for m in range(KT):
                        pq = [psum.tile([P, CH], F32, tag="mm", bufs=3,
                                        name="psq") for _ in range(NIC)]
                        for d_ in range(KT):
                            for ic in range(NIC):
                                nc.tensor.matmul(
                                    pq[ic], WqT[d_][:, m * P:(m + 1) * P],
                                    HcT[d_][:, ic * CH:(ic + 1) * CH],
                                    start=(d_ == 0), stop=(d_ == KT - 1))
                        for ic in range(NIC):
                            nc.vector.tensor_scalar(
                                QT[m][:, ic * CH:(ic + 1) * CH], pq[ic],
                                bq_sb[m], None, op0=ALU.add)
                    for k in range(KT):
                        pq2 = [psum.tile([P, CH], F32, tag="mm", bufs=3,
                                         name="psq2") for _ in range(NIC)]
                        for m in range(KT):
                            for ic in range(NIC):
                                nc.tensor.matmul(
                                    pq2[ic], Wk[m][:, k * P:(k + 1) * P],
                                    QT[m][:, ic * CH:(ic + 1) * CH],
                                    start=(m == 0), stop=(m == KT - 1))
                        for ic in range(NIC):
                            nc.vector.tensor_scalar(
                                q2f8[:, k:k + 1, ic * CH:(ic + 1) * CH],
                                pq2[ic], Q2SCALE, None, op0=ALU.mult)

                # -- distance groups, software pipelined ---------------------
                for jg in range(1, NG):
                    if jg + 1 < NG:
                        tts_next = load_group(jg + 1)
                        augg_next = xx_chain(jg + 1, tts_next)
                    else:
                        tts_next = augg_next = None
                    emit_ht8(2)
                    if jg == 1:
                        emit_tailw()
                    d2_group(jg, tts_cur, augg_cur)
                    tts_cur, augg_cur = tts_next, augg_next

            with tc.tile_pool(name="scl", bufs=1, space="DRAM") as dram:
                scl_dram = dram.tile([R, 1], F32, name="scldram")
                for it in range(NIT):
                    red = early.tile([P, 1], F32, name=f"red{it}")
                    nc.vector.reduce_sum(red, dsum[it],
                                         axis=mybir.AxisListType.X)
                    mcol = early.tile([P, 1], F32, name=f"mcol{it}")
                    nc.vector.tensor_scalar(mcol, red, 1.0 / N, None,
                                            op0=ALU.mult)
                    ucol = early.tile([P, 1], F32, name=f"ucol{it}")
                    nc.vector.reciprocal(ucol, mcol)
                    # measured mean of sqrt(dist2+16) = true mean + 8/m +
                    # diag excess 4/N; scale = 1/(1 + m - 8/m - 0.000488)
                    uc2 = early.tile([P, 1], F32, name=f"uc2{it}")
                    nc.vector.tensor_scalar(uc2, ucol, -8.0, 0.999512,
                                            op0=ALU.mult, op1=ALU.add)
                    tmp = early.tile([P, 1], F32, name=f"sctmp{it}")
                    nc.vector.tensor_add(tmp, mcol, uc2)
                    scol = early.tile([P, 1], F32, name=f"scol{it}")
                    nc.vector.reciprocal(scol, tmp)
                    nc.sync.dma_start(scl_dram[it * P:(it + 1) * P, :], scol)
                nc.sync.dma_start(
                    scl_row, scl_dram.rearrange("(a p) c -> a (p c)", a=1))
                nc.vector.tensor_copy(scl_b, scl_row)


        # ---- attention passes: pipelined logits(pair s) | G/rowsum(s-2) ----
        def attention_pass(ic, hooks=None):
            hooks = hooks or {}
            csl = slice(ic * CH, (ic + 1) * CH)
            g_ps = [psum.tile([P, CH], F32, tag=f"g{d_}", name=f"gps{d_}")
                    for d_ in range(KT)]
            rs_ps = psum.tile([P, CH], F32, tag="rowps", name="rsps")
            # two-deep pair pipeline: G/rowsum lag the logits by 2 pairs so
            # the exp of pair s-2 is long done when its G matmuls issue
            pipe = []  # [(e2_t, h2_t, s), ...]

            def g_mm(lag, k, stop=False):
                nc.tensor.matmul(g_ps[k], lag[1][:, :, k * P:(k + 1) * P],
                                 lag[0], start=(lag[2] == 0), stop=stop,
                                 perf_mode=DR)

            for s in range(NPAIR):
                h2_t = h_pool.tile([P, 2, D], F8, tag="h", name="h2t")
                nc.sync.dma_start(h2_t[:, 0:1, :],
                                  io["Hf8"][(2 * s) * P:(2 * s + 1) * P, :])
                nc.sync.dma_start(h2_t[:, 1:2, :],
                                  io["Hf8"][(2 * s + 1) * P:(2 * s + 2) * P, :])
                e2_t = e_pool.tile([P, 2, CH], F8, tag="e", name="e2t")
                st_a = psum.tile([P, CH], F32, tag="mm", bufs=3, name="sta")
                st_b = psum.tile([P, CH], F32, tag="mm", bufs=3, name="stb")
                lag = pipe[0] if len(pipe) == 2 else None
                nc.tensor.matmul(st_a, ht8[:, 0:2, (2 * s) * P:(2 * s + 1) * P],
                                 q2f8[:, 0:2, csl], start=True, stop=False,
                                 perf_mode=DR)
                if lag is not None:
                    g_mm(lag, 0)
                nc.tensor.matmul(st_a, ht8[:, 2:4, (2 * s) * P:(2 * s + 1) * P],
                                 q2f8[:, 2:4, csl], start=False, stop=True,
                                 perf_mode=DR)
                if lag is not None:
                    g_mm(lag, 1)
                nc.scalar.activation(e2_t[:, 0:1, :], st_a, AF.Exp,
                                     scale=EXPSCALE, bias=expb_col)
                nc.tensor.matmul(st_b,
                                 ht8[:, 0:2, (2 * s + 1) * P:(2 * s + 2) * P],
                                 q2f8[:, 0:2, csl], start=True, stop=False,
                                 perf_mode=DR)
                if lag is not None:
                    g_mm(lag, 2)
                nc.tensor.matmul(st_b,
                                 ht8[:, 2:4, (2 * s + 1) * P:(2 * s + 2) * P],
                                 q2f8[:, 2:4, csl], start=False, stop=True,
                                 perf_mode=DR)
                if lag is not None:
                    g_mm(lag, 3)
                    nc.tensor.matmul(rs_ps, onesw2, lag[0],
                                     start=(lag[2] == 0), stop=False,
                                     perf_mode=DR)
                    pipe.pop(0)
                nc.scalar.activation(e2_t[:, 1:2, :], st_b, AF.Exp,
                                     scale=EXPSCALE, bias=expb_col)
                pipe.append((e2_t, h2_t, s))
                if s in hooks:
                    hooks[s]()
            for (e2_t, h2_t, s) in pipe:
                last = s == NPAIR - 1
                for k in range(KT):
                    nc.tensor.matmul(g_ps[k], h2_t[:, :, k * P:(k + 1) * P],
                                     e2_t, start=(s == 0), stop=last,
                                     perf_mode=DR)
                nc.tensor.matmul(rs_ps, onesw2, e2_t, start=(s == 0),
                                 stop=last, perf_mode=DR)
            # drain accumulators promptly so the next pass can claim the banks
            for d_ in range(KT):
                nc.scalar.activation(GT[d_][:, csl], g_ps[d_], AF.Copy)
            nc.vector.tensor_copy(rs_row[0:1, csl], rs_ps[0:1, :])

        def tail_pre(ic):
            csl = slice(ic * CH, (ic + 1) * CH)
            nc.vector.reciprocal(sn_row[0:1, csl], rs_row[0:1, csl])
            nc.vector.tensor_mul(sn_row[0:1, csl], sn_row[0:1, csl],
                                 scl_row[0:1, csl])
            ps_snb = psum.tile([P, CH], F32, tag="mm", bufs=3, name="pssnb")
            nc.tensor.matmul(ps_snb, ones_f1, sn_row[0:1, csl],
                             start=True, stop=True)
            nc.vector.tensor_copy(SNB[:, csl], ps_snb)
            for d_ in range(KT):
                nc.vector.tensor_mul(GT[d_][:, csl], GT[d_][:, csl],
                                     SNB[:, csl])

        def tail_y(ic):
            csl = slice(ic * CH, (ic + 1) * CH)
            # Y^T = Wv Gn^T + (bv x scale): two m-chains in flight
            for m0 in range(0, KT, 2):
                py = [psum.tile([P, CH], F32, tag="mm", bufs=3, name="psy")
                      for _ in range(2)]
                for d_ in range(KT):
                    for u in range(2):
                        m = m0 + u
                        nc.tensor.matmul(py[u], WvT[d_][:, m * P:(m + 1) * P],
                                         GT[d_][:, csl],
                                         start=(d_ == 0), stop=False)
                for u in range(2):
                    m = m0 + u
                    nc.tensor.matmul(py[u], bv_row[0:1, m * P:(m + 1) * P],
                                     scl_b[0:1, csl], start=False, stop=True)
                for u in range(2):
                    m = m0 + u
                    nc.scalar.activation(YT[m][:, csl], py[u], AF.Copy)

        def tail_out(ic):
            csl = slice(ic * CH, (ic + 1) * CH)
            # out = Y Wo^T + bo for this chunk's 4 i-tiles, chains in pairs
            for it0 in range(ic * 4, (ic + 1) * 4, 2):
                po = [psum.tile([P, CH], F32, tag="mm", bufs=3, name="pso")
                      for _ in range(2)]
                for m in range(KT):
                    for u in range(2):
                        it = it0 + u
                        nc.tensor.matmul(po[u], YT[m][:, it * P:(it + 1) * P],
                                         WoT[m], start=(m == 0), stop=False)
                for u in range(2):
                    nc.tensor.matmul(po[u], ones_b1, bo_row,
                                     start=False, stop=True)
                for u in range(2):
                    it = it0 + u
                    o_t = o_pool.tile([P, D], F32, tag="o", name="ot")
                    nc.scalar.activation(o_t, po[u], AF.Copy)
                    nc.sync.dma_start(io["OUT"][it * P:(it + 1) * P, :], o_t)

        attention_pass(0)
        # pass-0 tail rides inside pass 1: its serial recip->snb->Y->out
        # chain hides behind pass-1's matmul stream
        attention_pass(1, hooks={6: lambda: tail_pre(0),
                                 14: lambda: tail_y(0),
                                 22: lambda: tail_out(0)})
        tail_pre(1)
        tail_y(1)
        tail_out(1)


_NC_CACHE = None


def _build():
    global _NC_CACHE
    if _NC_CACHE is not None:
        return _NC_CACHE
    nc = bacc.Bacc("TRN2", target_bir_lowering=False, debug=False,
                   enable_asserts=False, num_devices=NCORES)
    io = {
        "HTf8": nc.dram_tensor("HTf8", [D, N], F8, kind="ExternalInput").ap(),
        "Hf8": nc.dram_tensor("Hf8", [N, D], F8, kind="ExternalInput").ap(),
        "TTf8": nc.dram_tensor("TTf8", [D, N], F8, kind="ExternalInput").ap(),
        "TcTf8": nc.dram_tensor("TcTf8", [D, R], F8,
                                kind="ExternalInput").ap(),
        "HcTb": nc.dram_tensor("HcTb", [D, R], BF, kind="ExternalInput").ap(),
        "WqTb": nc.dram_tensor("WqTb", [D, D], BF, kind="ExternalInput").ap(),
        "Wkb": nc.dram_tensor("Wkb", [D, D], BF, kind="ExternalInput").ap(),
        "WvTb": nc.dram_tensor("WvTb", [D, D], BF, kind="ExternalInput").ap(),
        "WoTb": nc.dram_tensor("WoTb", [D, D], BF, kind="ExternalInput").ap(),
        "bqf": nc.dram_tensor("bqf", [D, 1], F32, kind="ExternalInput").ap(),
        "bvb": nc.dram_tensor("bvb", [1, D], BF, kind="ExternalInput").ap(),
        "bob": nc.dram_tensor("bob", [1, D], BF, kind="ExternalInput").ap(),
        "OUT": nc.dram_tensor("OUT", [R, D], F32, kind="ExternalOutput").ap(),
    }
    with tile.TileContext(nc) as tc:
        _emit(tc, io)
    nc.compile()
    _NC_CACHE = nc
    return nc


LAST_RESULTS = None


def _to_f8(a):
    return np.clip(a, -240.0, 240.0).astype(f8e4)


def kernel(H, T, Wq, bq, Wk, bk, Wv, bv, Wo, bo):
    global LAST_RESULTS
    H = np.ascontiguousarray(np.asarray(H, np.float32))
    T = np.ascontiguousarray(np.asarray(T, np.float32))

    HT = np.ascontiguousarray(H.T)
    TT = np.ascontiguousarray(T.T)
    HTb = HT.astype(bf16)
    shared = {
        "HTf8": _to_f8(HT),
        "Hf8": _to_f8(H),
        "TTf8": _to_f8(TT),
        "WqTb": np.ascontiguousarray(np.asarray(Wq, np.float32).T).astype(bf16),
        "Wkb": np.ascontiguousarray(np.asarray(Wk, np.float32)).astype(bf16),
        "WvTb": np.ascontiguousarray(np.asarray(Wv, np.float32).T).astype(bf16),
        "WoTb": np.ascontiguousarray(np.asarray(Wo, np.float32).T).astype(bf16),
        "bqf": np.asarray(bq, np.float32).reshape(D, 1).copy(),
        "bvb": np.asarray(bv, np.float32).reshape(1, D).astype(bf16),
        "bob": np.asarray(bo, np.float32).reshape(1, D).astype(bf16),
    }
    in_maps = []
    for c in range(NCORES):
        m = dict(shared)
        m["TcTf8"] = np.ascontiguousarray(shared["TTf8"][:, c * R:(c + 1) * R])
        m["HcTb"] = np.ascontiguousarray(HTb[:, c * R:(c + 1) * R])
        in_maps.append(m)

    nc = _build()
    res = bass_utils.run_bass_kernel_spmd(nc, in_maps,
                                          core_ids=list(range(NCORES)))
    LAST_RESULTS = res
    out = np.concatenate([res.results[c]["OUT"] for c in range(NCORES)],
                         axis=0)
    return np.ascontiguousarray(out.astype(np.float32))
